# revision 11
# baseline (speedup 1.0000x reference)
"""Cross-attention Trainium2 Bass kernel (v3: M-fold + fp8 DoubleRow).

Problem (per batch element, fp32):
    q = x1 @ Wq + bq; k = x2 @ Wk + bk; v = x2 @ Wv + bv
    out = softmax(q k^T / sqrt(512)) @ v        with LQ = LK = 2048, D = 512

Sharding: batch (B=8) across the 8 NeuronCores, one element per core.

Key algebra: q.k = x1 (Wq Wk^T) x2^T + [per-q const, drops in softmax]
             + beta[k] + [const, drops], with beta = x2 @ (Wk bq).
So the Q- and K-projections collapse into ONE projection K' = x2 @ MT
(MT = 32*(Wk Wq^T), host-computed) plus a cheap matvec beta; x1 enters the
scores matmul directly as host-quantized fp8 (no Q projection on device).

Precision plan (sim: 1.1e-2 max rel err vs 2e-2 gate):
  - K'-projection, beta, scores: fp8 DoubleRow matmuls (K=256/instruction,
    ~2x bf16 PE rate). x1, x2, MT host-quantized to fp8; K' requantized
    to fp8 pair-tiles on device.
  - V path: bf16 x2/Wv (fp8 V fails the error budget), V tiles in fp16.
  - P = exp(scores) in fp16: better accuracy than bf16 AND 2x DVE rate
    for the softmax-denominator accumulation.
  - PV: fp16 matmuls.
Schedule: beta interleaved with K' chains; V-projection chains interleaved
with qb0's scores (hides ACT exp); PV(qb) interleaves with front-loaded
scores(qb+1); denominators via ones-matmul + DRAM-bounce transpose.
"""
import sys

sys.path.insert(0, "/opt/trn_rl_repo")
import numpy as np
import ml_dtypes
import concourse.bass as bass
import concourse.tile as tile
import concourse.bacc as bacc
from concourse import mybir
from concourse.bass_utils import run_bass_kernel_spmd

B, LQ, LK, D = 8, 2048, 2048, 512
P = 128
NKT = LK // P          # 16 k-tiles
NDC = D // P           # 4 chunks of the contraction dim
NQB = LQ // 512        # 4 q-blocks of 512
NCORES = 8
SCALE = float(1.0 / np.sqrt(np.float32(D)))
QKS = 32.0             # fp8 range scale folded into MT
BETAS = 1024.0         # fp8 subnormal-avoidance scale on w2 (= Wk bq)

f32 = mybir.dt.float32
bf16 = mybir.dt.bfloat16
fp16 = mybir.dt.float16
fp8 = mybir.dt.float8e4
ts = bass.ts
Exp = mybir.ActivationFunctionType.Exp
DR = mybir.MatmulPerfMode.DoubleRow

_CACHE = {}


def _build():
    nc = bacc.Bacc("TRN2", target_bir_lowering=False, debug=False,
                   num_devices=NCORES)
    X1T8 = nc.declare_dram_parameter("x1t8", [D, LQ], fp8, isOutput=False)
    X2T8 = nc.declare_dram_parameter("x2t8", [D, LK], fp8, isOutput=False)
    X2T = nc.declare_dram_parameter("x2t", [D, LK], bf16, isOutput=False)
    MT8 = nc.declare_dram_parameter("mt8", [D, D], fp8, isOutput=False)
    WV = nc.declare_dram_parameter("wv", [D, D], bf16, isOutput=False)
    W28 = nc.declare_dram_parameter("w28", [P, 2, 2, 16], fp8, isOutput=False)
    BV = nc.declare_dram_parameter("bv", [D], f32, isOutput=False)
    OUT = nc.declare_dram_parameter("out", [LQ, D], f32, isOutput=True)
    DEN = nc.dram_tensor("den_scratch", [NQB, 512], f32)
    BSC = nc.dram_tensor("beta_scratch", [LK], f32)

    with tile.TileContext(nc) as tc:
        with (
            tc.tile_pool(name="const", bufs=1) as cpool,
            tc.tile_pool(name="wts", bufs=1) as wpool,
            tc.tile_pool(name="xts", bufs=1) as xts,
            tc.tile_pool(name="kp", bufs=1) as kpp,
            tc.tile_pool(name="vts", bufs=1) as vts,
            tc.tile_pool(name="ptp", bufs=32) as ptp,
            tc.tile_pool(name="accp", bufs=2) as accp,
            tc.tile_pool(name="obuf", bufs=2) as obuf,
            tc.tile_pool(name="psW", bufs=1, space="PSUM") as psW,
            tc.tile_pool(name="psB", bufs=3, space="PSUM") as psB,
            tc.tile_pool(name="psO", bufs=3, space="PSUM") as psO,
            tc.tile_pool(name="psD", bufs=1, space="PSUM") as psD,
        ):
            # ---- startup DMAs, fp8 critical path first ----
            # MT8 pairs [128, 2, 512] (pair = adjacent 128-deep b-chunks)
            mtp = [wpool.tile([P, 2, D], fp8, tag=f"mtp{j}", name=f"mtp{j}")
                   for j in range(2)]
            mt_src = MT8.ap().rearrange("(j i p) a -> j p i a", j=2, i=2)
            x2p = [xts.tile([P, 2, LK], fp8, tag=f"x2p{j}", name=f"x2p{j}")
                   for j in range(2)]
            x2_src8 = X2T8.ap().rearrange("(j i p) k -> j p i k", j=2, i=2)
            x1p = [xts.tile([P, 2, LQ], fp8, tag=f"x1p{j}", name=f"x1p{j}")
                   for j in range(2)]
            x1_src = X1T8.ap().rearrange("(j i p) q -> j p i q", j=2, i=2)

            for j in range(2):
                eng = nc.sync if j == 0 else nc.scalar
                eng.dma_start(mtp[j][:], mt_src[j])
            for j in range(2):
                eng = nc.sync if j == 0 else nc.scalar
                eng.dma_start(x2p[j][:, :, ts(0, 512)],
                              x2_src8[j][:, :, ts(0, 512)])
            w28 = cpool.tile([P, 2, 2, 16], fp8, tag="w28")
            nc.sync.dma_start(w28[:], W28[:])
            bv_f = cpool.tile([1, D], f32, tag="bv_f")
            nc.scalar.dma_start(bv_f[:], BV[:].unsqueeze(0))
            # x1p qb0 right away (scores(0) starts ~15us)
            for j in range(2):
                eng = nc.scalar if j == 0 else nc.sync
                eng.dma_start(x1p[j][:, :, ts(0, 512)],
                              x1_src[j][:, :, ts(0, 512)])
            for kb in range(1, 4):
                for j in range(2):
                    eng = nc.sync if (kb + j) % 2 == 0 else nc.scalar
                    eng.dma_start(x2p[j][:, :, ts(kb, 512)],
                                  x2_src8[j][:, :, ts(kb, 512)])
            # V-path operands next: wv split across queues, then bf16 x2
            wv = wpool.tile([P, NDC, D], bf16, tag="wv", name="w_wv")
            wv_src = WV.ap().rearrange("(c p) n -> p c n", p=P)
            nc.sync.dma_start(wv[:, 0:2], wv_src[:, 0:2])
            nc.scalar.dma_start(wv[:, 2:4], wv_src[:, 2:4])
            x2t = [xts.tile([P, LK], bf16, tag=f"x2t{ci}", name=f"x2t{ci}")
                   for ci in range(NDC)]
            for ci in range(NDC):
                eng = nc.sync if ci % 2 == 0 else nc.scalar
                eng.dma_start(x2t[ci][:], X2T[ts(ci, P), :])
            # x1p qb1-3 last (needed from ~38us on)
            for qb in (1, 2, 3):
                for j in range(2):
                    eng = nc.sync if (qb + j) % 2 == 0 else nc.scalar
                    eng.dma_start(x1p[j][:, :, ts(qb, 512)],
                                  x1_src[j][:, :, ts(qb, 512)])

            # persistent K' fp8 pairs and V fp16
            kp8 = [kpp.tile([P, 2, LK], fp8, tag=f"kp{j}", name=f"kp{j}")
                   for j in range(2)]
            vt = [vts.tile([P, D], fp16, tag=f"v{t}", name=f"v{t}")
                  for t in range(NKT)]

            # ---- small constants ----
            ones2h = cpool.tile([P, 2], fp16, tag="ones2h")
            nc.vector.memset(ones2h[:], 1.0)
            onesr_b = cpool.tile([1, P], bf16, tag="onesr_b")
            nc.vector.memset(onesr_b[:], 1.0)
            bv_b = cpool.tile([1, D], bf16, tag="bv_b")
            nc.vector.tensor_copy(bv_b[:], bv_f[:])

            # ---- PE warm-up: un-throttle HAM during the input DMA wait ----
            warm_a = cpool.tile([P, P], bf16, tag="warm_a")
            nc.vector.memset(warm_a[:], 0.125)
            warm_b = cpool.tile([P, 512], bf16, tag="warm_b")
            nc.vector.memset(warm_b[:], 0.125)
            wps = psW.tile([P, 512], f32, tag="warm", name="warm_ps")
            for _ in range(16):
                nc.tensor.matmul(wps[:], warm_a[:], warm_b[:],
                                 start=True, stop=True)

            # ---------------- phase A: beta + K' (all fp8 DR) ----------------
            beta_sb = cpool.tile([1, LK], f32, tag="beta_sb")

            def emit_beta(kb):
                bps = psD.tile([2, 512], f32, tag="d", name=f"beta_{kb}")
                for j in range(2):
                    nc.tensor.matmul(bps[:], w28[:, j, :, 0:2],
                                     x2p[j][:, :, ts(kb, 512)],
                                     start=(j == 0), stop=(j == 1),
                                     perf_mode=DR)
                # undo the fp8-subnormal-avoidance scale on w2
                nc.scalar.mul(beta_sb[:, ts(kb, 512)], bps[0:1, :],
                              1.0 / BETAS)

            def emit_kp(kb, ci):
                mm = psB.tile([P, 512], f32, tag="mm")
                for j in range(2):
                    nc.tensor.matmul(mm[:], mtp[j][:, :, ts(ci, P)],
                                     x2p[j][:, :, ts(kb, 512)],
                                     start=(j == 0), stop=(j == 1),
                                     perf_mode=DR)
                nc.vector.tensor_copy(kp8[ci // 2][:, ci % 2, ts(kb, 512)],
                                      mm[:])

            for kb in range(4):
                emit_beta(kb)
                for ci in range(NDC):
                    emit_kp(kb, ci)
                if kb == 0:
                    # broadcast bv to 128 partitions via PE (K=1 ones x bv);
                    # reuses the warmup PSUM bank (same tag)
                    bvb_ps = psW.tile([P, 512], f32, tag="warm",
                                      name="bvb_ps")
                    nc.tensor.matmul(bvb_ps[:], onesr_b[:], bv_b[:],
                                     start=True, stop=True)
                    bv_bcast = cpool.tile([P, D], f32, tag="bv_bcast")
                    nc.vector.tensor_copy(bv_bcast[:], bvb_ps[:])

            # beta row -> DRAM -> per-partition columns [128, 16]
            nc.scalar.dma_start(BSC[:].unsqueeze(0), beta_sb[:])
            bcols = cpool.tile([P, NKT], f32, tag="bcols")
            nc.scalar.dma_start(bcols[:],
                                BSC.ap().rearrange("(t p) -> p t", p=P))

            # ---------------- phase B helpers ----------------
            def emit_v(t):
                mm = psB.tile([P, 512], f32, tag="mm")
                for cj in range(NDC):
                    nc.tensor.matmul(mm[:], x2t[cj][:, ts(t, P)],
                                     wv[:, cj, :], start=(cj == 0),
                                     stop=(cj == NDC - 1))
                nc.vector.tensor_add(vt[t][:], mm[:], bv_bcast[:])

            acc_cur = {}

            def emit_score(qb, t, pts, acc):
                smm = psB.tile([P, 512], f32, tag="mm")
                for j in range(2):
                    nc.tensor.matmul(smm[:], kp8[j][:, :, ts(t, P)],
                                     x1p[j][:, :, ts(qb, 512)],
                                     start=(j == 0), stop=(j == 1),
                                     perf_mode=DR)
                ptile = ptp.tile([P, 512], fp16, tag="pt")
                nc.scalar.activation(ptile[:], smm[:], Exp,
                                     scale=SCALE / QKS,
                                     bias=bcols[:, t:t + 1])
                pts.append(ptile)
                if t == 0:
                    nc.vector.tensor_copy(acc[0][:], ptile[:])
                else:
                    nc.vector.tensor_add(acc[t % 2][:], acc[(t + 1) % 2][:],
                                         ptile[:])

            def start_scores(qb):
                pts = []
                acc = [accp.tile([P, 512], fp16, tag="accA", name=f"accA{qb}"),
                       accp.tile([P, 512], fp16, tag="accB", name=f"accB{qb}")]
                acc_cur[qb] = (pts, acc)
                return pts, acc

            def emit_den(qb):
                pts, acc = acc_cur[qb]
                dps = psD.tile([2, 512], f32, tag="d", name=f"den_{qb}")
                nc.tensor.matmul(dps[:], ones2h[:], acc[(NKT - 1) % 2][:],
                                 start=True, stop=True)
                den_sb = cpool.tile([1, 512], f32, tag="den_sb",
                                    name=f"den_sb_{qb}")
                nc.vector.tensor_copy(den_sb[:], dps[0:1, :])
                nc.scalar.dma_start(DEN[qb].unsqueeze(0), den_sb[:])
                den_cols = obuf.tile([P, 4], f32, tag="den_cols")
                nc.scalar.dma_start(
                    den_cols[:], DEN[qb].rearrange("(s p) -> p s", p=P))
                rec = obuf.tile([P, 4], f32, tag="rec", name=f"rec_{qb}")
                nc.vector.reciprocal(rec[:], den_cols[:])
                return rec

            def emit_pv(qb, s, pts, rec):
                ops = psO.tile([P, 512], f32, tag="o")
                for t in range(NKT):
                    nc.tensor.matmul(ops[:], pts[t][:, ts(s, P)], vt[t][:],
                                     start=(t == 0), stop=(t == NKT - 1))
                osb = obuf.tile([P, 512], f32, tag="osb")
                nc.vector.tensor_scalar_mul(osb[:], ops[:], rec[:, s:s + 1])
                eng = nc.sync if s % 2 == 0 else nc.scalar
                eng.dma_start(OUT[ts(qb * 4 + s, P), :], osb[:])

            # ---------------- phase B ----------------
            # qb0 scores interleaved with V-projection (hides ACT exp);
            # V starts at t=4 so its bf16 operands have DMA slack
            pts0, acc0 = start_scores(0)
            for t in range(NKT):
                emit_score(0, t, pts0, acc0)
                if t >= 4:
                    emit_v(t - 4)
            for t in range(NKT - 4, NKT):
                emit_v(t)
            rec = emit_den(0)

            prev_pts = pts0
            for qb in range(NQB):
                if qb + 1 < NQB:
                    # front-load next qb's scores into the first two PV
                    # sub-blocks so den(qb+1) has slack before PV(qb+1)
                    next_pts, next_acc = start_scores(qb + 1)
                    for s in range(4):
                        emit_pv(qb, s, prev_pts, rec)
                        if s < 2:
                            for t in range(8 * s, 8 * s + 8):
                                emit_score(qb + 1, t, next_pts, next_acc)
                    rec = emit_den(qb + 1)
                    prev_pts = next_pts
                else:
                    for s in range(4):
                        emit_pv(qb, s, prev_pts, rec)

    nc.compile()
    return nc


def _get_nc():
    if "nc" not in _CACHE:
        _CACHE["nc"] = _build()
    return _CACHE["nc"]


def kernel(x_1, x_2, Wq, bq, Wk, bk, Wv, bv, **_run_kwargs):
    bf = ml_dtypes.bfloat16
    f8 = ml_dtypes.float8_e4m3
    x_1t8 = np.ascontiguousarray(
        np.asarray(x_1, dtype=np.float32).transpose(0, 2, 1)).astype(f8)
    x_2tf = np.ascontiguousarray(
        np.asarray(x_2, dtype=np.float32).transpose(0, 2, 1))
    x_2t8 = x_2tf.astype(f8)
    x_2t = x_2tf.astype(bf)
    Wq = np.asarray(Wq, dtype=np.float32)
    Wk = np.asarray(Wk, dtype=np.float32)
    Wv_b = np.ascontiguousarray(np.asarray(Wv, dtype=np.float32).astype(bf))
    bq = np.asarray(bq, dtype=np.float32)
    bv = np.ascontiguousarray(np.asarray(bv, dtype=np.float32))

    # MT = 32 * (Wk Wq^T) [b, a] in fp8; w2 = SCALE*1024*(Wk bq) packed
    # [128, jpair, ipair, dup] in fp8
    MT8 = np.ascontiguousarray((Wk @ Wq.T) * QKS).astype(f8)
    w2 = ((Wk @ bq) * SCALE * BETAS).astype(np.float32)
    # dup dim padded to 16 so the DR pair stride is 16B-aligned
    w2p = np.ascontiguousarray(
        np.repeat(w2.reshape(2, 2, P).transpose(2, 0, 1)[:, :, :, None],
                  16, axis=3)).astype(f8)

    nc = _get_nc()
    in_maps = [
        {"x1t8": x_1t8[c], "x2t8": x_2t8[c], "x2t": x_2t[c], "mt8": MT8,
         "wv": Wv_b, "w28": w2p, "bv": bv}
        for c in range(NCORES)
    ]
    res = run_bass_kernel_spmd(nc, in_maps, list(range(NCORES)),
                               **_run_kwargs)
    if _run_kwargs:
        _CACHE["last_results"] = res
    return np.stack([res.results[c]["out"] for c in range(NCORES)])


# revision 12
# speedup vs baseline: 1.0461x; 1.0461x over previous
"""Cross-attention Trainium2 Bass kernel (v3: M-fold + fp8 DoubleRow).

Problem (per batch element, fp32):
    q = x1 @ Wq + bq; k = x2 @ Wk + bk; v = x2 @ Wv + bv
    out = softmax(q k^T / sqrt(512)) @ v        with LQ = LK = 2048, D = 512

Sharding: batch (B=8) across the 8 NeuronCores, one element per core.

Key algebra: q.k = x1 (Wq Wk^T) x2^T + [per-q const, drops in softmax]
             + beta[k] + [const, drops], with beta = x2 @ (Wk bq).
So the Q- and K-projections collapse into ONE projection K' = x2 @ MT
(MT = 32*(Wk Wq^T), host-computed) plus a cheap matvec beta; x1 enters the
scores matmul directly as host-quantized fp8 (no Q projection on device).

Precision plan (sim: 1.1e-2 max rel err vs 2e-2 gate):
  - K'-projection, beta, scores: fp8 DoubleRow matmuls (K=256/instruction,
    ~2x bf16 PE rate). x1, x2, MT host-quantized to fp8; K' requantized
    to fp8 pair-tiles on device.
  - V path: bf16 x2/Wv (fp8 V fails the error budget), V tiles in fp16.
  - P = exp(scores) in fp16: better accuracy than bf16 AND 2x DVE rate
    for the softmax-denominator accumulation.
  - PV: fp16 matmuls.
Schedule: beta interleaved with K' chains; V-projection chains interleaved
with qb0's scores (hides ACT exp); PV(qb) interleaves with front-loaded
scores(qb+1); denominators via ones-matmul + DRAM-bounce transpose.
"""
import sys

sys.path.insert(0, "/opt/trn_rl_repo")
import numpy as np
import ml_dtypes
import concourse.bass as bass
import concourse.tile as tile
import concourse.bacc as bacc
from concourse import mybir
from concourse.bass_utils import run_bass_kernel_spmd

B, LQ, LK, D = 8, 2048, 2048, 512
P = 128
NKT = LK // P          # 16 k-tiles
NDC = D // P           # 4 chunks of the contraction dim
NQB = LQ // 512        # 4 q-blocks of 512
NCORES = 8
SCALE = float(1.0 / np.sqrt(np.float32(D)))
QKS = 32.0             # fp8 range scale folded into MT
BETAS = 1024.0         # fp8 subnormal-avoidance scale on w2 (= Wk bq)

f32 = mybir.dt.float32
bf16 = mybir.dt.bfloat16
fp16 = mybir.dt.float16
fp8 = mybir.dt.float8e4
ts = bass.ts
Exp = mybir.ActivationFunctionType.Exp
DR = mybir.MatmulPerfMode.DoubleRow

_CACHE = {}


def _build():
    nc = bacc.Bacc("TRN2", target_bir_lowering=False, debug=False,
                   num_devices=NCORES)
    X1T8 = nc.declare_dram_parameter("x1t8", [D, LQ], fp8, isOutput=False)
    X2T8 = nc.declare_dram_parameter("x2t8", [D, LK], fp8, isOutput=False)
    X2T = nc.declare_dram_parameter("x2t", [D, LK], bf16, isOutput=False)
    MT8 = nc.declare_dram_parameter("mt8", [D, D], fp8, isOutput=False)
    WV = nc.declare_dram_parameter("wv", [D, D], bf16, isOutput=False)
    W28 = nc.declare_dram_parameter("w28", [P, 2, 2, 16], fp8, isOutput=False)
    BV = nc.declare_dram_parameter("bv", [D], f32, isOutput=False)
    OUT = nc.declare_dram_parameter("out", [LQ, D], f32, isOutput=True)
    DEN = nc.dram_tensor("den_scratch", [NQB, 512], f32)
    BSC = nc.dram_tensor("beta_scratch", [LK], f32)

    with tile.TileContext(nc) as tc:
        with (
            tc.tile_pool(name="const", bufs=1) as cpool,
            tc.tile_pool(name="wts", bufs=1) as wpool,
            tc.tile_pool(name="xts", bufs=1) as xts,
            tc.tile_pool(name="kp", bufs=1) as kpp,
            tc.tile_pool(name="vts", bufs=1) as vts,
            tc.tile_pool(name="ptp", bufs=32) as ptp,
            tc.tile_pool(name="accp", bufs=2) as accp,
            tc.tile_pool(name="obuf", bufs=2) as obuf,
            tc.tile_pool(name="psW", bufs=1, space="PSUM") as psW,
            tc.tile_pool(name="psB", bufs=4, space="PSUM") as psB,
            tc.tile_pool(name="psO", bufs=2, space="PSUM") as psO,
            tc.tile_pool(name="psD", bufs=1, space="PSUM") as psD,
        ):
            # ---- startup DMAs, fp8 critical path first ----
            # MT8 pairs [128, 2, 512] (pair = adjacent 128-deep b-chunks)
            mtp = [wpool.tile([P, 2, D], fp8, tag=f"mtp{j}", name=f"mtp{j}")
                   for j in range(2)]
            mt_src = MT8.ap().rearrange("(j i p) a -> j p i a", j=2, i=2)
            x2p = [xts.tile([P, 2, LK], fp8, tag=f"x2p{j}", name=f"x2p{j}")
                   for j in range(2)]
            x2_src8 = X2T8.ap().rearrange("(j i p) k -> j p i k", j=2, i=2)
            x1p = [xts.tile([P, 2, LQ], fp8, tag=f"x1p{j}", name=f"x1p{j}")
                   for j in range(2)]
            x1_src = X1T8.ap().rearrange("(j i p) q -> j p i q", j=2, i=2)

            # scalar queue stays LIGHT so phase-B ACT work (exp) is not
            # stuck behind DMA backlog (queues are in-order)
            nc.sync.dma_start(mtp[0][:], mt_src[0])
            nc.scalar.dma_start(mtp[1][:], mt_src[1])
            nc.sync.dma_start(x2p[0][:, :, ts(0, 512)],
                              x2_src8[0][:, :, ts(0, 512)])
            nc.scalar.dma_start(x2p[1][:, :, ts(0, 512)],
                              x2_src8[1][:, :, ts(0, 512)])
            w28 = cpool.tile([P, 2, 2, 16], fp8, tag="w28")
            nc.sync.dma_start(w28[:], W28[:])
            bv_f = cpool.tile([1, D], f32, tag="bv_f")
            nc.scalar.dma_start(bv_f[:], BV[:].unsqueeze(0))
            # x1p qb0 right away (scores(0) starts ~15us)
            nc.sync.dma_start(x1p[0][:, :, ts(0, 512)],
                              x1_src[0][:, :, ts(0, 512)])
            nc.scalar.dma_start(x1p[1][:, :, ts(0, 512)],
                              x1_src[1][:, :, ts(0, 512)])
            for kb in range(1, 4):
                nc.sync.dma_start(x2p[0][:, :, ts(kb, 512)],
                                  x2_src8[0][:, :, ts(kb, 512)])
                nc.scalar.dma_start(x2p[1][:, :, ts(kb, 512)],
                                    x2_src8[1][:, :, ts(kb, 512)])
            # V-path operands: wv + 2 x2t tiles on scalar (done ~15us),
            # rest on sync
            wv = wpool.tile([P, NDC, D], bf16, tag="wv", name="w_wv")
            wv_src = WV.ap().rearrange("(c p) n -> p c n", p=P)
            nc.sync.dma_start(wv[:, 0:2], wv_src[:, 0:2])
            nc.scalar.dma_start(wv[:, 2:4], wv_src[:, 2:4])
            x2t = [xts.tile([P, LK], bf16, tag=f"x2t{ci}", name=f"x2t{ci}")
                   for ci in range(NDC)]
            for ci in range(NDC):
                eng = nc.sync if ci < 2 else nc.scalar
                eng.dma_start(x2t[ci][:], X2T[ts(ci, P), :])
            # x1p qb1-3 last, all on sync (needed from ~38us on)
            for qb in (1, 2, 3):
                for j in range(2):
                    nc.sync.dma_start(x1p[j][:, :, ts(qb, 512)],
                                      x1_src[j][:, :, ts(qb, 512)])

            # persistent K' fp8 pairs and V fp16
            kp8 = [kpp.tile([P, 2, LK], fp8, tag=f"kp{j}", name=f"kp{j}")
                   for j in range(2)]
            vt = [vts.tile([P, D], fp16, tag=f"v{t}", name=f"v{t}")
                  for t in range(NKT)]

            # ---- small constants ----
            ones2h = cpool.tile([P, 2], fp16, tag="ones2h")
            nc.vector.memset(ones2h[:], 1.0)
            onesr_b = cpool.tile([1, P], bf16, tag="onesr_b")
            nc.vector.memset(onesr_b[:], 1.0)
            bv_b = cpool.tile([1, D], bf16, tag="bv_b")
            nc.vector.tensor_copy(bv_b[:], bv_f[:])

            # ---- PE warm-up: un-throttle HAM during the input DMA wait ----
            warm_a = cpool.tile([P, P], bf16, tag="warm_a")
            nc.vector.memset(warm_a[:], 0.125)
            warm_b = cpool.tile([P, 512], bf16, tag="warm_b")
            nc.vector.memset(warm_b[:], 0.125)
            wps = psW.tile([P, 512], f32, tag="warm", name="warm_ps")
            for _ in range(16):
                nc.tensor.matmul(wps[:], warm_a[:], warm_b[:],
                                 start=True, stop=True)

            # ---------------- phase A: beta + K' (all fp8 DR) ----------------
            beta_sb = cpool.tile([1, LK], f32, tag="beta_sb")
            binv = cpool.tile([1, 1], f32, tag="binv")
            nc.vector.memset(binv[:], 1.0 / BETAS)

            def emit_beta(kb):
                bps = psD.tile([2, 512], f32, tag="d", name=f"beta_{kb}")
                for j in range(2):
                    nc.tensor.matmul(bps[:], w28[:, j, :, 0:2],
                                     x2p[j][:, :, ts(kb, 512)],
                                     start=(j == 0), stop=(j == 1),
                                     perf_mode=DR)
                # undo the fp8-subnormal-avoidance scale on w2 (on DVE:
                # the scalar queue is full of startup DMAs at this point)
                nc.vector.tensor_scalar_mul(beta_sb[:, ts(kb, 512)],
                                            bps[0:1, :], binv[:])

            def emit_kp(kb, ci):
                mm = psB.tile([P, 512], f32, tag="mm")
                for j in range(2):
                    nc.tensor.matmul(mm[:], mtp[j][:, :, ts(ci, P)],
                                     x2p[j][:, :, ts(kb, 512)],
                                     start=(j == 0), stop=(j == 1),
                                     perf_mode=DR)
                nc.vector.tensor_copy(kp8[ci // 2][:, ci % 2, ts(kb, 512)],
                                      mm[:])

            for kb in range(4):
                emit_beta(kb)
                for ci in range(NDC):
                    emit_kp(kb, ci)
                if kb == 0:
                    # broadcast bv to 128 partitions via PE (K=1 ones x bv);
                    # reuses the warmup PSUM bank (same tag)
                    bvb_ps = psW.tile([P, 512], f32, tag="warm",
                                      name="bvb_ps")
                    nc.tensor.matmul(bvb_ps[:], onesr_b[:], bv_b[:],
                                     start=True, stop=True)
                    bv_bcast = cpool.tile([P, D], f32, tag="bv_bcast")
                    nc.vector.tensor_copy(bv_bcast[:], bvb_ps[:])

            # beta row -> DRAM -> per-partition columns [128, 16]
            nc.scalar.dma_start(BSC[:].unsqueeze(0), beta_sb[:])
            bcols = cpool.tile([P, NKT], f32, tag="bcols")
            nc.scalar.dma_start(bcols[:],
                                BSC.ap().rearrange("(t p) -> p t", p=P))

            # ---------------- phase B helpers ----------------
            def emit_v(t):
                mm = psB.tile([P, 512], f32, tag="mm")
                for cj in range(NDC):
                    nc.tensor.matmul(mm[:], x2t[cj][:, ts(t, P)],
                                     wv[:, cj, :], start=(cj == 0),
                                     stop=(cj == NDC - 1))
                nc.vector.tensor_add(vt[t][:], mm[:], bv_bcast[:])

            acc_cur = {}

            def emit_score(qb, t, pts, acc):
                smm = psB.tile([P, 512], f32, tag="mm")
                for j in range(2):
                    nc.tensor.matmul(smm[:], kp8[j][:, :, ts(t, P)],
                                     x1p[j][:, :, ts(qb, 512)],
                                     start=(j == 0), stop=(j == 1),
                                     perf_mode=DR)
                ptile = ptp.tile([P, 512], fp16, tag="pt")
                nc.scalar.activation(ptile[:], smm[:], Exp,
                                     scale=SCALE / QKS,
                                     bias=bcols[:, t:t + 1])
                pts.append(ptile)
                if t == 0:
                    nc.vector.tensor_copy(acc[0][:], ptile[:])
                else:
                    nc.vector.tensor_add(acc[t % 2][:], acc[(t + 1) % 2][:],
                                         ptile[:])

            def start_scores(qb):
                pts = []
                acc = [accp.tile([P, 512], fp16, tag="accA", name=f"accA{qb}"),
                       accp.tile([P, 512], fp16, tag="accB", name=f"accB{qb}")]
                acc_cur[qb] = (pts, acc)
                return pts, acc

            def emit_den(qb):
                pts, acc = acc_cur[qb]
                dps = psD.tile([2, 512], f32, tag="d", name=f"den_{qb}")
                nc.tensor.matmul(dps[:], ones2h[:], acc[(NKT - 1) % 2][:],
                                 start=True, stop=True)
                den_sb = cpool.tile([1, 512], f32, tag="den_sb",
                                    name=f"den_sb_{qb}")
                nc.vector.tensor_copy(den_sb[:], dps[0:1, :])
                nc.scalar.dma_start(DEN[qb].unsqueeze(0), den_sb[:])
                den_cols = obuf.tile([P, 4], f32, tag="den_cols")
                nc.scalar.dma_start(
                    den_cols[:], DEN[qb].rearrange("(s p) -> p s", p=P))
                rec = obuf.tile([P, 4], f32, tag="rec", name=f"rec_{qb}")
                nc.vector.reciprocal(rec[:], den_cols[:])
                return rec

            def emit_pv(qb, s, pts, rec):
                ops = psO.tile([P, 512], f32, tag="o")
                for t in range(NKT):
                    nc.tensor.matmul(ops[:], pts[t][:, ts(s, P)], vt[t][:],
                                     start=(t == 0), stop=(t == NKT - 1))
                osb = obuf.tile([P, 512], f32, tag="osb")
                nc.vector.tensor_scalar_mul(osb[:], ops[:], rec[:, s:s + 1])
                eng = nc.sync if s % 2 == 0 else nc.scalar
                eng.dma_start(OUT[ts(qb * 4 + s, P), :], osb[:])

            # ---------------- phase B ----------------
            # qb0 scores interleaved with V-projection (hides ACT exp);
            # V starts at t=4 so its bf16 operands have DMA slack
            pts0, acc0 = start_scores(0)
            for t in range(NKT):
                emit_score(0, t, pts0, acc0)
                if t >= 4:
                    emit_v(t - 4)
            for t in range(NKT - 4, NKT):
                emit_v(t)
            rec = emit_den(0)

            prev_pts = pts0
            for qb in range(NQB):
                if qb + 1 < NQB:
                    # front-load next qb's scores into the first two PV
                    # sub-blocks so den(qb+1) has slack before PV(qb+1)
                    next_pts, next_acc = start_scores(qb + 1)
                    for s in range(4):
                        emit_pv(qb, s, prev_pts, rec)
                        if s < 2:
                            for t in range(8 * s, 8 * s + 8):
                                emit_score(qb + 1, t, next_pts, next_acc)
                    rec = emit_den(qb + 1)
                    prev_pts = next_pts
                else:
                    for s in range(4):
                        emit_pv(qb, s, prev_pts, rec)

    nc.compile()
    return nc


def _get_nc():
    if "nc" not in _CACHE:
        _CACHE["nc"] = _build()
    return _CACHE["nc"]


def kernel(x_1, x_2, Wq, bq, Wk, bk, Wv, bv, **_run_kwargs):
    bf = ml_dtypes.bfloat16
    f8 = ml_dtypes.float8_e4m3
    x_1t8 = np.ascontiguousarray(
        np.asarray(x_1, dtype=np.float32).transpose(0, 2, 1)).astype(f8)
    x_2tf = np.ascontiguousarray(
        np.asarray(x_2, dtype=np.float32).transpose(0, 2, 1))
    x_2t8 = x_2tf.astype(f8)
    x_2t = x_2tf.astype(bf)
    Wq = np.asarray(Wq, dtype=np.float32)
    Wk = np.asarray(Wk, dtype=np.float32)
    Wv_b = np.ascontiguousarray(np.asarray(Wv, dtype=np.float32).astype(bf))
    bq = np.asarray(bq, dtype=np.float32)
    bv = np.ascontiguousarray(np.asarray(bv, dtype=np.float32))

    # MT = 32 * (Wk Wq^T) [b, a] in fp8; w2 = SCALE*1024*(Wk bq) packed
    # [128, jpair, ipair, dup] in fp8
    MT8 = np.ascontiguousarray((Wk @ Wq.T) * QKS).astype(f8)
    w2 = ((Wk @ bq) * SCALE * BETAS).astype(np.float32)
    # dup dim padded to 16 so the DR pair stride is 16B-aligned
    w2p = np.ascontiguousarray(
        np.repeat(w2.reshape(2, 2, P).transpose(2, 0, 1)[:, :, :, None],
                  16, axis=3)).astype(f8)

    nc = _get_nc()
    in_maps = [
        {"x1t8": x_1t8[c], "x2t8": x_2t8[c], "x2t": x_2t[c], "mt8": MT8,
         "wv": Wv_b, "w28": w2p, "bv": bv}
        for c in range(NCORES)
    ]
    res = run_bass_kernel_spmd(nc, in_maps, list(range(NCORES)),
                               **_run_kwargs)
    if _run_kwargs:
        _CACHE["last_results"] = res
    return np.stack([res.results[c]["out"] for c in range(NCORES)])


# revision 14
# speedup vs baseline: 1.1069x; 1.0581x over previous
"""Cross-attention Trainium2 Bass kernel (v3: M-fold + fp8 DoubleRow).

Problem (per batch element, fp32):
    q = x1 @ Wq + bq; k = x2 @ Wk + bk; v = x2 @ Wv + bv
    out = softmax(q k^T / sqrt(512)) @ v        with LQ = LK = 2048, D = 512

Sharding: batch (B=8) across the 8 NeuronCores, one element per core.

Key algebra: q.k = x1 (Wq Wk^T) x2^T + [per-q const, drops in softmax]
             + beta[k] + [const, drops], with beta = x2 @ (Wk bq).
So the Q- and K-projections collapse into ONE projection K' = x2 @ MT
(MT = 32*(Wk Wq^T), host-computed) plus a cheap matvec beta; x1 enters the
scores matmul directly as host-quantized fp8 (no Q projection on device).

Precision plan (sim: 1.1e-2 max rel err vs 2e-2 gate):
  - K'-projection, beta, scores: fp8 DoubleRow matmuls (K=256/instruction,
    ~2x bf16 PE rate). x1, x2, MT host-quantized to fp8; K' requantized
    to fp8 pair-tiles on device.
  - V path: bf16 x2/Wv (fp8 V fails the error budget), V tiles in fp16.
  - P = exp(scores) in fp16: better accuracy than bf16 AND 2x DVE rate
    for the softmax-denominator accumulation.
  - PV: fp16 matmuls.
Schedule: beta interleaved with K' chains; V-projection chains interleaved
with qb0's scores (hides ACT exp); PV(qb) interleaves with front-loaded
scores(qb+1); denominators via ones-matmul + DRAM-bounce transpose.
"""
import sys

sys.path.insert(0, "/opt/trn_rl_repo")
import numpy as np
import ml_dtypes
import concourse.bass as bass
import concourse.tile as tile
import concourse.bacc as bacc
from concourse import mybir
from concourse.bass_utils import run_bass_kernel_spmd

B, LQ, LK, D = 8, 2048, 2048, 512
P = 128
NKT = LK // P          # 16 k-tiles
NDC = D // P           # 4 chunks of the contraction dim
NQB = LQ // 512        # 4 q-blocks of 512
NCORES = 8
SCALE = float(1.0 / np.sqrt(np.float32(D)))
QKS = 32.0             # fp8 range scale folded into MT
BETAS = 1024.0         # fp8 subnormal-avoidance scale on w2 (= Wk bq)

f32 = mybir.dt.float32
bf16 = mybir.dt.bfloat16
fp16 = mybir.dt.float16
fp8 = mybir.dt.float8e4
ts = bass.ts
Exp = mybir.ActivationFunctionType.Exp
DR = mybir.MatmulPerfMode.DoubleRow

_CACHE = {}


def _build():
    nc = bacc.Bacc("TRN2", target_bir_lowering=False, debug=False,
                   num_devices=NCORES)
    X1T8 = nc.declare_dram_parameter("x1t8", [D, LQ], fp8, isOutput=False)
    X2T8 = nc.declare_dram_parameter("x2t8", [D, LK], fp8, isOutput=False)
    X2T = nc.declare_dram_parameter("x2t", [D, LK], bf16, isOutput=False)
    MT8 = nc.declare_dram_parameter("mt8", [D, D], fp8, isOutput=False)
    WV = nc.declare_dram_parameter("wv", [D, D], bf16, isOutput=False)
    W28 = nc.declare_dram_parameter("w28", [P, 2, 2, 16], fp8, isOutput=False)
    BV = nc.declare_dram_parameter("bv", [D], f32, isOutput=False)
    OUT = nc.declare_dram_parameter("out", [LQ, D], f32, isOutput=True)

    with tile.TileContext(nc) as tc:
        with (
            tc.tile_pool(name="const", bufs=1) as cpool,
            tc.tile_pool(name="wts", bufs=1) as wpool,
            tc.tile_pool(name="xts", bufs=1) as xts,
            tc.tile_pool(name="kp", bufs=1) as kpp,
            tc.tile_pool(name="vts", bufs=1) as vts,
            tc.tile_pool(name="ptp", bufs=32) as ptp,
            tc.tile_pool(name="accp", bufs=2) as accp,
            tc.tile_pool(name="obuf", bufs=2) as obuf,
            tc.tile_pool(name="psW", bufs=1, space="PSUM") as psW,
            tc.tile_pool(name="psB", bufs=3, space="PSUM") as psB,
            tc.tile_pool(name="psO", bufs=3, space="PSUM") as psO,
            tc.tile_pool(name="psD", bufs=1, space="PSUM") as psD,
        ):
            # ---- startup DMAs, fp8 critical path first ----
            # MT8 pairs [128, 2, 512] (pair = adjacent 128-deep b-chunks)
            mtp = [wpool.tile([P, 2, D], fp8, tag=f"mtp{j}", name=f"mtp{j}")
                   for j in range(2)]
            mt_src = MT8.ap().rearrange("(j i p) a -> j p i a", j=2, i=2)
            x2p = [xts.tile([P, 2, LK], fp8, tag=f"x2p{j}", name=f"x2p{j}")
                   for j in range(2)]
            x2_src8 = X2T8.ap().rearrange("(j i p) k -> j p i k", j=2, i=2)
            x1p = [xts.tile([P, 2, LQ], fp8, tag=f"x1p{j}", name=f"x1p{j}")
                   for j in range(2)]
            x1_src = X1T8.ap().rearrange("(j i p) q -> j p i q", j=2, i=2)

            # scalar queue stays LIGHT so phase-B ACT work (exp) is not
            # stuck behind DMA backlog (queues are in-order)
            nc.sync.dma_start(mtp[0][:], mt_src[0])
            nc.scalar.dma_start(mtp[1][:], mt_src[1])
            nc.sync.dma_start(x2p[0][:, :, ts(0, 512)],
                              x2_src8[0][:, :, ts(0, 512)])
            nc.scalar.dma_start(x2p[1][:, :, ts(0, 512)],
                              x2_src8[1][:, :, ts(0, 512)])
            w28 = cpool.tile([P, 2, 2, 16], fp8, tag="w28")
            nc.sync.dma_start(w28[:], W28[:])
            bv_f = cpool.tile([1, D], f32, tag="bv_f")
            nc.scalar.dma_start(bv_f[:], BV[:].unsqueeze(0))
            # x1p qb0 right away (scores(0) starts ~15us)
            nc.sync.dma_start(x1p[0][:, :, ts(0, 512)],
                              x1_src[0][:, :, ts(0, 512)])
            nc.scalar.dma_start(x1p[1][:, :, ts(0, 512)],
                              x1_src[1][:, :, ts(0, 512)])
            for kb in range(1, 4):
                nc.sync.dma_start(x2p[0][:, :, ts(kb, 512)],
                                  x2_src8[0][:, :, ts(kb, 512)])
                nc.scalar.dma_start(x2p[1][:, :, ts(kb, 512)],
                                    x2_src8[1][:, :, ts(kb, 512)])
            # V-path operands: wv + 2 x2t tiles on scalar (done ~15us),
            # rest on sync
            wv = wpool.tile([P, NDC, D], bf16, tag="wv", name="w_wv")
            wv_src = WV.ap().rearrange("(c p) n -> p c n", p=P)
            nc.sync.dma_start(wv[:, 0:2], wv_src[:, 0:2])
            nc.scalar.dma_start(wv[:, 2:4], wv_src[:, 2:4])
            x2t = [xts.tile([P, LK], bf16, tag=f"x2t{ci}", name=f"x2t{ci}")
                   for ci in range(NDC)]
            for ci in range(NDC):
                eng = nc.sync if ci < 2 else nc.scalar
                eng.dma_start(x2t[ci][:], X2T[ts(ci, P), :])
            # x1p qb1-3 last, all on sync (needed from ~38us on)
            for qb in (1, 2, 3):
                for j in range(2):
                    nc.sync.dma_start(x1p[j][:, :, ts(qb, 512)],
                                      x1_src[j][:, :, ts(qb, 512)])

            # persistent K' fp8 pairs and V fp16
            kp8 = [kpp.tile([P, 2, LK], fp8, tag=f"kp{j}", name=f"kp{j}")
                   for j in range(2)]
            vt = [vts.tile([P, D], fp16, tag=f"v{t}", name=f"v{t}")
                  for t in range(NKT)]

            # ---- small constants (memsets first: the DVE queue is
            # in-order and the warmup matmuls wait on warm_a/warm_b) ----
            warm_a = cpool.tile([P, P], bf16, tag="warm_a")
            nc.vector.memset(warm_a[:], 0.125)
            warm_b = cpool.tile([P, 512], bf16, tag="warm_b")
            nc.vector.memset(warm_b[:], 0.125)
            ones1h = cpool.tile([P, 1], fp16, tag="ones1h")
            nc.vector.memset(ones1h[:], 1.0)
            onesr_b = cpool.tile([1, P], bf16, tag="onesr_b")
            nc.vector.memset(onesr_b[:], 1.0)
            idf = cpool.tile([1, 1], f32, tag="idf")
            nc.vector.memset(idf[:], 1.0)

            # ---- PE warm-up: un-throttle HAM during the input DMA wait ----
            wps = psW.tile([P, 512], f32, tag="warm", name="warm_ps")
            for _ in range(10):
                nc.tensor.matmul(wps[:], warm_a[:], warm_b[:],
                                 start=True, stop=True)
            bv_b = cpool.tile([1, D], bf16, tag="bv_b")
            nc.vector.tensor_copy(bv_b[:], bv_f[:])

            # ---------------- phase A: beta + K' (all fp8 DR) ----------------
            beta_sb = cpool.tile([1, LK], f32, tag="beta_sb")
            binv = cpool.tile([1, 1], f32, tag="binv")
            nc.vector.memset(binv[:], 1.0 / BETAS)

            def emit_beta(kb):
                bps = psD.tile([2, 512], f32, tag="d", name=f"beta_{kb}")
                for j in range(2):
                    nc.tensor.matmul(bps[:], w28[:, j, :, 0:2],
                                     x2p[j][:, :, ts(kb, 512)],
                                     start=(j == 0), stop=(j == 1),
                                     perf_mode=DR)
                # undo the fp8-subnormal-avoidance scale on w2 (on DVE:
                # the scalar queue is full of startup DMAs at this point)
                nc.vector.tensor_scalar_mul(beta_sb[:, ts(kb, 512)],
                                            bps[0:1, :], binv[:])

            def emit_kp(kb, ci):
                mm = psB.tile([P, 512], f32, tag="mm")
                for j in range(2):
                    nc.tensor.matmul(mm[:], mtp[j][:, :, ts(ci, P)],
                                     x2p[j][:, :, ts(kb, 512)],
                                     start=(j == 0), stop=(j == 1),
                                     perf_mode=DR)
                nc.vector.tensor_copy(kp8[ci // 2][:, ci % 2, ts(kb, 512)],
                                      mm[:])

            for kb in range(4):
                emit_beta(kb)
                for ci in range(NDC):
                    emit_kp(kb, ci)
                if kb == 0:
                    # broadcast bv to 128 partitions via PE (K=1 ones x bv);
                    # reuses the warmup PSUM bank (same tag)
                    bvb_ps = psW.tile([P, 512], f32, tag="warm",
                                      name="bvb_ps")
                    nc.tensor.matmul(bvb_ps[:], onesr_b[:], bv_b[:],
                                     start=True, stop=True)
                    bv_bcast = cpool.tile([P, D], f32, tag="bv_bcast")
                    nc.vector.tensor_copy(bv_bcast[:], bvb_ps[:])

            # beta row -> per-partition columns via 16 PE transposes
            # ([1,128] -> [128,1]); avoids a DRAM bounce (DMA queue order
            # does not imply transfer-completion order -> race)
            btp = psW.tile([P, 512], f32, tag="warm", name="betaT_ps")
            for t in range(NKT):
                nc.tensor.transpose(btp[:, t:t + 1], beta_sb[:, ts(t, P)],
                                    idf[:])
            bcols = cpool.tile([P, NKT], f32, tag="bcols")
            nc.vector.tensor_copy(bcols[:], btp[:, 0:NKT])

            # ---------------- phase B helpers ----------------
            def emit_v(t):
                mm = psB.tile([P, 512], f32, tag="mm")
                for cj in range(NDC):
                    nc.tensor.matmul(mm[:], x2t[cj][:, ts(t, P)],
                                     wv[:, cj, :], start=(cj == 0),
                                     stop=(cj == NDC - 1))
                nc.vector.tensor_add(vt[t][:], mm[:], bv_bcast[:])

            acc_cur = {}

            def emit_score(qb, t, pts, acc):
                smm = psB.tile([P, 512], f32, tag="mm")
                for j in range(2):
                    nc.tensor.matmul(smm[:], kp8[j][:, :, ts(t, P)],
                                     x1p[j][:, :, ts(qb, 512)],
                                     start=(j == 0), stop=(j == 1),
                                     perf_mode=DR)
                ptile = ptp.tile([P, 512], fp16, tag="pt")
                nc.scalar.activation(ptile[:], smm[:], Exp,
                                     scale=SCALE / QKS,
                                     bias=bcols[:, t:t + 1])
                pts.append(ptile)
                if t == 0:
                    nc.vector.tensor_copy(acc[0][:], ptile[:])
                else:
                    nc.vector.tensor_add(acc[t % 2][:], acc[(t + 1) % 2][:],
                                         ptile[:])

            def start_scores(qb):
                pts = []
                acc = [accp.tile([P, 512], fp16, tag="accA", name=f"accA{qb}"),
                       accp.tile([P, 512], fp16, tag="accB", name=f"accB{qb}")]
                acc_cur[qb] = (pts, acc)
                return pts, acc

            def emit_den(qb):
                pts, acc = acc_cur[qb]
                af = acc[(NKT - 1) % 2]
                # den^T directly: [128q,1] = acc-slice.T @ ones column
                dt = psW.tile([P, 512], f32, tag="warm", name=f"dent_{qb}")
                for s in range(4):
                    nc.tensor.matmul(dt[:, s:s + 1], af[:, ts(s, P)],
                                     ones1h[:], start=True, stop=True)
                rec = obuf.tile([P, 4], f32, tag="rec", name=f"rec_{qb}")
                nc.vector.reciprocal(rec[:], dt[:, 0:4])
                return rec

            def emit_pv(qb, s, pts, rec):
                ops = psO.tile([P, 512], f32, tag="o")
                for t in range(NKT):
                    nc.tensor.matmul(ops[:], pts[t][:, ts(s, P)], vt[t][:],
                                     start=(t == 0), stop=(t == NKT - 1))
                osb = obuf.tile([P, 512], f32, tag="osb")
                nc.vector.tensor_scalar_mul(osb[:], ops[:], rec[:, s:s + 1])
                eng = nc.sync if s % 2 == 0 else nc.scalar
                eng.dma_start(OUT[ts(qb * 4 + s, P), :], osb[:])

            # ---------------- phase B ----------------
            # qb0 scores interleaved with V-projection (hides ACT exp);
            # V starts at t=4 so its bf16 operands have DMA slack
            pts0, acc0 = start_scores(0)
            for t in range(NKT):
                emit_score(0, t, pts0, acc0)
                if t >= 1:
                    emit_v(t - 1)
            emit_v(NKT - 1)
            rec = emit_den(0)

            prev_pts = pts0
            for qb in range(NQB):
                if qb + 1 < NQB:
                    # front-load next qb's scores into the first two PV
                    # sub-blocks so den(qb+1) has slack before PV(qb+1)
                    next_pts, next_acc = start_scores(qb + 1)
                    for s in range(4):
                        emit_pv(qb, s, prev_pts, rec)
                        if s < 2:
                            for t in range(8 * s, 8 * s + 8):
                                emit_score(qb + 1, t, next_pts, next_acc)
                    rec = emit_den(qb + 1)
                    prev_pts = next_pts
                else:
                    for s in range(4):
                        emit_pv(qb, s, prev_pts, rec)

    nc.compile()
    return nc


def _get_nc():
    if "nc" not in _CACHE:
        _CACHE["nc"] = _build()
    return _CACHE["nc"]


def kernel(x_1, x_2, Wq, bq, Wk, bk, Wv, bv, **_run_kwargs):
    bf = ml_dtypes.bfloat16
    f8 = ml_dtypes.float8_e4m3
    x_1t8 = np.ascontiguousarray(
        np.asarray(x_1, dtype=np.float32).transpose(0, 2, 1)).astype(f8)
    x_2tf = np.ascontiguousarray(
        np.asarray(x_2, dtype=np.float32).transpose(0, 2, 1))
    x_2t8 = x_2tf.astype(f8)
    x_2t = x_2tf.astype(bf)
    Wq = np.asarray(Wq, dtype=np.float32)
    Wk = np.asarray(Wk, dtype=np.float32)
    Wv_b = np.ascontiguousarray(np.asarray(Wv, dtype=np.float32).astype(bf))
    bq = np.asarray(bq, dtype=np.float32)
    bv = np.ascontiguousarray(np.asarray(bv, dtype=np.float32))

    # MT = 32 * (Wk Wq^T) [b, a] in fp8; w2 = SCALE*1024*(Wk bq) packed
    # [128, jpair, ipair, dup] in fp8
    MT8 = np.ascontiguousarray((Wk @ Wq.T) * QKS).astype(f8)
    w2 = ((Wk @ bq) * SCALE * BETAS).astype(np.float32)
    # dup dim padded to 16 so the DR pair stride is 16B-aligned
    w2p = np.ascontiguousarray(
        np.repeat(w2.reshape(2, 2, P).transpose(2, 0, 1)[:, :, :, None],
                  16, axis=3)).astype(f8)

    nc = _get_nc()
    in_maps = [
        {"x1t8": x_1t8[c], "x2t8": x_2t8[c], "x2t": x_2t[c], "mt8": MT8,
         "wv": Wv_b, "w28": w2p, "bv": bv}
        for c in range(NCORES)
    ]
    res = run_bass_kernel_spmd(nc, in_maps, list(range(NCORES)),
                               **_run_kwargs)
    if _run_kwargs:
        _CACHE["last_results"] = res
    return np.stack([res.results[c]["out"] for c in range(NCORES)])
